# revision 15
# baseline (speedup 1.0000x reference)
"""A8W4 Llama MLP (gate/up/down with SwiGLU + requant) on 8 TRN2 NeuronCores.

Strategy: data-parallel over the token dim. Each core gets T/8 = 512 tokens
and the full (replicated) weight set; there are no collectives. The host
pre-tiles all operands into SBUF-layout order so every DMA is a contiguous
partition-major block, and gathers/transposes the 8 output shards at the end.

All matmuls run in bf16: the int8 activations and int4 weights are exactly
representable in bf16, and every partial sum stays below 2^24, so fp32 PSUM
accumulation reproduces the reference integer arithmetic exactly.
"""

import sys

sys.path.insert(0, "/opt/trn_rl_repo")

import numpy as np
import ml_dtypes

import concourse.mybir as mybir
from concourse import bacc
from concourse.tile import TileContext
from concourse.bass_utils import run_bass_kernel_spmd


def _install_ntff_hook():
    # Optional: lets trace=True / BASS_TRACE=1 capture neuron-profile NTFFs
    # under axon (the container's antenv stub lacks axon_hooks).
    try:
        import types
        import antenv
        if "antenv.axon_hooks" in sys.modules:
            return
        if "/root/.axon_site" not in sys.path:
            sys.path.insert(0, "/root/.axon_site")
        from trn_agent_boot.trn_boot import _ntff_profile_via_ctypes
        hook = _ntff_profile_via_ctypes("/opt/axon/libaxon_pjrt.so")
        m = types.ModuleType("antenv.axon_hooks")
        m._hook = hook
        m.set_axon_ntff_profile_hook = lambda h: setattr(m, "_hook", h)
        m.get_axon_ntff_profile_hook = lambda: m._hook
        sys.modules["antenv.axon_hooks"] = m
        antenv.axon_hooks = m
    except Exception:
        pass


_install_ntff_hook()

T, H, I = 4096, 4096, 11008
NCORES = 8
TC = T // NCORES      # 512 tokens per core = matmul free dim
HT = H // 128         # 32 contraction tiles for gate/up
IT = I // 128         # 86 intermediate tiles
OT = H // 128         # 32 output tiles for down
DHALF = IT // 2       # 43: down-weight chunk size (i-tiles per DMA)

BF16 = mybir.dt.bfloat16
F32 = mybir.dt.float32
I8 = mybir.dt.int8

_cached_nc = None


def _build():
    nc = bacc.Bacc("TRN2", target_bir_lowering=False, debug=False, num_devices=NCORES)

    xT = nc.declare_dram_parameter("xT", [128, HT, TC], BF16, isOutput=False)
    gw = nc.declare_dram_parameter("gw", [IT, 128, HT, 128], BF16, isOutput=False)
    uw = nc.declare_dram_parameter("uw", [IT, 128, HT, 128], BF16, isOutput=False)
    dw = nc.declare_dram_parameter("dw", [OT, 2, 128, DHALF, 128], BF16, isOutput=False)
    ga = nc.declare_dram_parameter("ga", [128, IT], F32, isOutput=False)
    gb = nc.declare_dram_parameter("gb", [128, IT], F32, isOutput=False)
    ua = nc.declare_dram_parameter("ua", [128, IT], F32, isOutput=False)
    ub = nc.declare_dram_parameter("ub", [128, IT], F32, isOutput=False)
    da = nc.declare_dram_parameter("da", [128, OT], F32, isOutput=False)
    db = nc.declare_dram_parameter("db", [128, OT], F32, isOutput=False)
    out = nc.declare_dram_parameter("out", [OT, 128, TC], F32, isOutput=True)

    SILU = mybir.ActivationFunctionType.Silu
    IDENT = mybir.ActivationFunctionType.Identity

    with TileContext(nc) as tc:
        with (
            tc.tile_pool(name="xp", bufs=1) as xp,
            tc.tile_pool(name="hqp", bufs=1) as hqp,
            tc.tile_pool(name="scp", bufs=1) as scp,
            tc.tile_pool(name="gp", bufs=2) as gp,
            tc.tile_pool(name="upool", bufs=2) as upool,
            tc.tile_pool(name="dpool", bufs=2) as dpool,
            tc.tile_pool(name="tmp", bufs=2) as tmp,
            tc.tile_pool(name="tmp8", bufs=2) as tmp8,
            tc.tile_pool(name="outp", bufs=2) as outp,
            tc.tile_pool(name="psg", bufs=3, space="PSUM") as psg,
            tc.tile_pool(name="psu", bufs=3, space="PSUM") as psu,
            tc.tile_pool(name="psd", bufs=2, space="PSUM") as psd,
        ):
            # x as independent tiles so matmul deps are per-chunk; a small
            # first chunk goes on the fast HWDGE (sync) queue so MM0 starts
            # early, the rest stream on gpsimd in parallel with the weight
            # DMAs on sync
            XCHUNKS = [4, 4, 8, 8, 8]
            x_map = {}
            hc = 0
            for ci, w_c in enumerate(XCHUNKS):
                xt_c = xp.tile([128, w_c, TC], BF16, tag=f"x{ci}")
                eng = nc.sync if ci == 0 else nc.gpsimd
                eng.dma_start(xt_c[:], xT[:, hc:hc + w_c, :])
                for j in range(w_c):
                    x_map[hc + j] = xt_c[:, j, :]
                hc += w_c

            ga_sb = scp.tile([128, IT], F32, tag="ga")
            nc.gpsimd.dma_start(ga_sb[:], ga[:, :])
            gb_sb = scp.tile([128, IT], F32, tag="gb")
            nc.gpsimd.dma_start(gb_sb[:], gb[:, :])
            ua_sb = scp.tile([128, IT], F32, tag="ua")
            nc.gpsimd.dma_start(ua_sb[:], ua[:, :])
            ub_sb = scp.tile([128, IT], F32, tag="ub")
            nc.gpsimd.dma_start(ub_sb[:], ub[:, :])
            da_sb = scp.tile([128, OT], F32, tag="da")
            nc.gpsimd.dma_start(da_sb[:], da[:, :])
            db_sb = scp.tile([128, OT], F32, tag="db")
            nc.gpsimd.dma_start(db_sb[:], db[:, :])

            hq_sb = hqp.tile([128, IT, TC], BF16)

            # Phase 1: g/u projections + SwiGLU + requant, one i-tile at a time
            for it in range(IT):
                gw_t = gp.tile([128, HT, 128], BF16, tag="gw")
                if it == 0:
                    # split the very first weight DMA so MM0 starts early
                    nc.sync.dma_start(gw_t[:, :4, :], gw[it, :, :4, :])
                    nc.sync.dma_start(gw_t[:, 4:, :], gw[it, :, 4:, :])
                else:
                    nc.sync.dma_start(gw_t[:], gw[it])
                uw_t = upool.tile([128, HT, 128], BF16, tag="uw")
                nc.sync.dma_start(uw_t[:], uw[it])

                pg = psg.tile([128, TC], F32, tag="pg")
                for ho in range(HT):
                    nc.tensor.matmul(pg[:], gw_t[:, ho, :], x_map[ho],
                                     start=(ho == 0), stop=(ho == HT - 1))
                pu = psu.tile([128, TC], F32, tag="pu")
                for ho in range(HT):
                    nc.tensor.matmul(pu[:], uw_t[:, ho, :], x_map[ho],
                                     start=(ho == 0), stop=(ho == HT - 1))

                sg = tmp.tile([128, TC], F32, tag="sg")
                nc.scalar.activation(sg[:], pg[:], SILU,
                                     bias=gb_sb[:, it:it + 1], scale=ga_sb[:, it:it + 1])
                su = tmp.tile([128, TC], F32, tag="su")
                nc.scalar.activation(su[:], pu[:], IDENT,
                                     bias=ub_sb[:, it:it + 1], scale=ua_sb[:, it:it + 1])
                # round+saturate to int8 via the DVE output converter, then
                # widen to bf16 for the down matmul
                h8 = tmp8.tile([128, TC], I8, tag="h8")
                nc.vector.tensor_mul(h8[:], sg[:], su[:])
                nc.vector.tensor_copy(hq_sb[:, it, :], h8[:])

            # Phase 2: down projection, one o-tile at a time
            for ot in range(OT):
                pd = psd.tile([128, TC], F32, tag="pd")
                for half in range(2):
                    dw_t = dpool.tile([128, DHALF, 128], BF16, tag="dw")
                    nc.sync.dma_start(dw_t[:], dw[ot, half])
                    for k in range(DHALF):
                        it = half * DHALF + k
                        nc.tensor.matmul(pd[:], dw_t[:, k, :], hq_sb[:, it, :],
                                         start=(it == 0), stop=(it == IT - 1))
                o_sb = outp.tile([128, TC], F32, tag="o")
                nc.scalar.activation(o_sb[:], pd[:], IDENT,
                                     bias=db_sb[:, ot:ot + 1], scale=da_sb[:, ot:ot + 1])
                nc.sync.dma_start(out[ot], o_sb[:])

    nc.finalize()
    return nc


def _prep_inputs(x, gate_w, up_w, down_w, gate_alpha, gate_bias, up_alpha, up_bias,
                 down_alpha, down_bias, down_input_scale):
    bf16 = ml_dtypes.bfloat16

    # per-core activations: xT[hi, ho, t] = x[c*TC + t, ho*128 + hi]
    xTs = []
    for c in range(NCORES):
        xc = np.asarray(x[c * TC:(c + 1) * TC], dtype=np.float32)
        xTs.append(np.ascontiguousarray(
            xc.T.reshape(HT, 128, TC).transpose(1, 0, 2)).astype(bf16))

    # gate/up: w[it, hi, ho, ii] = W[it*128 + ii, ho*128 + hi]
    def prep_gu(w):
        w4 = np.asarray(w, dtype=np.float32).reshape(IT, 128, HT, 128)
        return np.ascontiguousarray(w4.transpose(0, 3, 2, 1)).astype(bf16)

    gw_h = prep_gu(gate_w)
    uw_h = prep_gu(up_w)

    # down: dw[ot, half, ii, k, oi] = down_w[ot*128 + oi, (half*DHALF + k)*128 + ii]
    d5 = np.asarray(down_w, dtype=np.float32).reshape(OT, 128, 2, DHALF, 128)
    dw_h = np.ascontiguousarray(d5.transpose(0, 2, 4, 3, 1)).astype(bf16)

    def perch(v, nt):
        return np.ascontiguousarray(
            np.asarray(v, dtype=np.float64).reshape(nt, 128).T).astype(np.float32)

    s = float(np.asarray(down_input_scale, dtype=np.float64))
    ga_h = perch(gate_alpha, IT)
    gb_h = perch(gate_bias, IT)
    ua_h = perch(np.asarray(up_alpha, dtype=np.float64) / s, IT)
    ub_h = perch(np.asarray(up_bias, dtype=np.float64) / s, IT)
    da_h = perch(down_alpha, OT)
    db_h = perch(down_bias, OT)

    in_maps = []
    for c in range(NCORES):
        in_maps.append(dict(xT=xTs[c], gw=gw_h, uw=uw_h, dw=dw_h,
                            ga=ga_h, gb=gb_h, ua=ua_h, ub=ub_h,
                            da=da_h, db=db_h))
    return in_maps


last_result = None


def kernel(**inputs):
    global _cached_nc, last_result
    if _cached_nc is None:
        _cached_nc = _build()
    in_maps = _prep_inputs(**inputs)
    res = run_bass_kernel_spmd(_cached_nc, in_maps, core_ids=list(range(NCORES)))
    last_result = res
    cols = [res.results[c]["out"].reshape(H, TC) for c in range(NCORES)]
    full = np.concatenate(cols, axis=1)  # [H, T]
    return np.ascontiguousarray(full.T).astype(np.float32)  # [T, H]


# revision 16
# speedup vs baseline: 1.1964x; 1.1964x over previous
"""A8W4 Llama MLP (gate/up/down with SwiGLU + requant) on 8 TRN2 NeuronCores.

Strategy: data-parallel over the token dim. Each core gets T/8 = 512 tokens
and the full (replicated) weight set; there are no collectives. The host
pre-tiles all operands into SBUF-layout order so every DMA is a contiguous
partition-major block, and gathers/transposes the 8 output shards at the end.

All matmuls run in bf16: the int8 activations and int4 weights are exactly
representable in bf16, and every partial sum stays below 2^24, so fp32 PSUM
accumulation reproduces the reference integer arithmetic exactly.
"""

import sys

sys.path.insert(0, "/opt/trn_rl_repo")

import numpy as np
import ml_dtypes

import concourse.mybir as mybir
from concourse import bacc
from concourse.tile import TileContext
from concourse.bass_utils import run_bass_kernel_spmd


def _install_ntff_hook():
    # Optional: lets trace=True / BASS_TRACE=1 capture neuron-profile NTFFs
    # under axon (the container's antenv stub lacks axon_hooks).
    try:
        import types
        import antenv
        if "antenv.axon_hooks" in sys.modules:
            return
        if "/root/.axon_site" not in sys.path:
            sys.path.insert(0, "/root/.axon_site")
        from trn_agent_boot.trn_boot import _ntff_profile_via_ctypes
        hook = _ntff_profile_via_ctypes("/opt/axon/libaxon_pjrt.so")
        m = types.ModuleType("antenv.axon_hooks")
        m._hook = hook
        m.set_axon_ntff_profile_hook = lambda h: setattr(m, "_hook", h)
        m.get_axon_ntff_profile_hook = lambda: m._hook
        sys.modules["antenv.axon_hooks"] = m
        antenv.axon_hooks = m
    except Exception:
        pass


_install_ntff_hook()

T, H, I = 4096, 4096, 11008
NCORES = 8
TC = T // NCORES      # 512 tokens per core = matmul free dim
HT = H // 128         # 32 contraction tiles for gate/up
IT = I // 128         # 86 intermediate tiles
OT = H // 128         # 32 output tiles for down
DHALF = IT // 2       # 43: down-weight chunk size (i-tiles per DMA)

BF16 = mybir.dt.bfloat16
F32 = mybir.dt.float32
I8 = mybir.dt.int8

_cached_nc = None


def _build():
    nc = bacc.Bacc("TRN2", target_bir_lowering=False, debug=False, num_devices=NCORES)

    xT = nc.declare_dram_parameter("xT", [128, HT, TC], BF16, isOutput=False)
    gw = nc.declare_dram_parameter("gw", [IT, 128, HT, 128], BF16, isOutput=False)
    uw = nc.declare_dram_parameter("uw", [IT, 128, HT, 128], BF16, isOutput=False)
    dw = nc.declare_dram_parameter("dw", [OT, 2, 128, DHALF, 128], BF16, isOutput=False)
    ga = nc.declare_dram_parameter("ga", [128, IT], F32, isOutput=False)
    gb = nc.declare_dram_parameter("gb", [128, IT], F32, isOutput=False)
    ua = nc.declare_dram_parameter("ua", [128, IT], F32, isOutput=False)
    ub = nc.declare_dram_parameter("ub", [128, IT], F32, isOutput=False)
    da = nc.declare_dram_parameter("da", [128, OT], F32, isOutput=False)
    db = nc.declare_dram_parameter("db", [128, OT], F32, isOutput=False)
    out = nc.declare_dram_parameter("out", [OT, 128, TC], F32, isOutput=True)

    SILU = mybir.ActivationFunctionType.Silu
    IDENT = mybir.ActivationFunctionType.Identity

    with TileContext(nc) as tc:
        with (
            tc.tile_pool(name="xp", bufs=1) as xp,
            tc.tile_pool(name="hqp", bufs=1) as hqp,
            tc.tile_pool(name="scp", bufs=1) as scp,
            tc.tile_pool(name="gp", bufs=2) as gp,
            tc.tile_pool(name="upool", bufs=2) as upool,
            tc.tile_pool(name="dpool", bufs=2) as dpool,
            tc.tile_pool(name="tmp", bufs=2) as tmp,
            tc.tile_pool(name="tmp8", bufs=2) as tmp8,
            tc.tile_pool(name="outp", bufs=2) as outp,
            tc.tile_pool(name="psg", bufs=3, space="PSUM") as psg,
            tc.tile_pool(name="psu", bufs=3, space="PSUM") as psu,
            tc.tile_pool(name="psd", bufs=2, space="PSUM") as psd,
        ):
            # x as independent tiles so matmul deps are per-chunk; a small
            # first chunk goes on the fast HWDGE (sync) queue so MM0 starts
            # early, the rest stream on gpsimd in parallel with the weight
            # DMAs on sync
            XCHUNKS = [4, 4, 8, 8, 8]
            x_map = {}
            hc = 0
            for ci, w_c in enumerate(XCHUNKS):
                xt_c = xp.tile([128, w_c, TC], BF16, tag=f"x{ci}")
                eng = nc.sync if ci <= 1 else nc.gpsimd
                eng.dma_start(xt_c[:], xT[:, hc:hc + w_c, :])
                for j in range(w_c):
                    x_map[hc + j] = xt_c[:, j, :]
                hc += w_c

            ga_sb = scp.tile([128, IT], F32, tag="ga")
            nc.gpsimd.dma_start(ga_sb[:], ga[:, :])
            gb_sb = scp.tile([128, IT], F32, tag="gb")
            nc.gpsimd.dma_start(gb_sb[:], gb[:, :])
            ua_sb = scp.tile([128, IT], F32, tag="ua")
            nc.gpsimd.dma_start(ua_sb[:], ua[:, :])
            ub_sb = scp.tile([128, IT], F32, tag="ub")
            nc.gpsimd.dma_start(ub_sb[:], ub[:, :])
            da_sb = scp.tile([128, OT], F32, tag="da")
            nc.gpsimd.dma_start(da_sb[:], da[:, :])
            db_sb = scp.tile([128, OT], F32, tag="db")
            nc.gpsimd.dma_start(db_sb[:], db[:, :])

            hq_sb = hqp.tile([128, IT, TC], BF16)

            # Phase 1: g/u projections + SwiGLU + requant, one i-tile at a time
            for it in range(IT):
                gw_t = gp.tile([128, HT, 128], BF16, tag="gw")
                if it == 0:
                    # split the very first weight DMA so MM0 starts early
                    nc.sync.dma_start(gw_t[:, :4, :], gw[it, :, :4, :])
                    nc.sync.dma_start(gw_t[:, 4:, :], gw[it, :, 4:, :])
                else:
                    nc.sync.dma_start(gw_t[:], gw[it])
                uw_t = upool.tile([128, HT, 128], BF16, tag="uw")
                nc.sync.dma_start(uw_t[:], uw[it])

                pg = psg.tile([128, TC], F32, tag="pg")
                for ho in range(HT):
                    nc.tensor.matmul(pg[:], gw_t[:, ho, :], x_map[ho],
                                     start=(ho == 0), stop=(ho == HT - 1))
                pu = psu.tile([128, TC], F32, tag="pu")
                for ho in range(HT):
                    nc.tensor.matmul(pu[:], uw_t[:, ho, :], x_map[ho],
                                     start=(ho == 0), stop=(ho == HT - 1))

                sg = tmp.tile([128, TC], F32, tag="sg")
                nc.scalar.activation(sg[:], pg[:], SILU,
                                     bias=gb_sb[:, it:it + 1], scale=ga_sb[:, it:it + 1])
                su = tmp.tile([128, TC], F32, tag="su")
                nc.scalar.activation(su[:], pu[:], IDENT,
                                     bias=ub_sb[:, it:it + 1], scale=ua_sb[:, it:it + 1])
                # round+saturate to int8 via the DVE output converter, then
                # widen to bf16 for the down matmul
                h8 = tmp8.tile([128, TC], I8, tag="h8")
                nc.vector.tensor_mul(h8[:], sg[:], su[:])
                nc.vector.tensor_copy(hq_sb[:, it, :], h8[:])

            # Phase 2: down projection, one o-tile at a time
            for ot in range(OT):
                pd = psd.tile([128, TC], F32, tag="pd")
                for half in range(2):
                    dw_t = dpool.tile([128, DHALF, 128], BF16, tag="dw")
                    nc.sync.dma_start(dw_t[:], dw[ot, half])
                    for k in range(DHALF):
                        it = half * DHALF + k
                        nc.tensor.matmul(pd[:], dw_t[:, k, :], hq_sb[:, it, :],
                                         start=(it == 0), stop=(it == IT - 1))
                o_sb = outp.tile([128, TC], F32, tag="o")
                nc.scalar.activation(o_sb[:], pd[:], IDENT,
                                     bias=db_sb[:, ot:ot + 1], scale=da_sb[:, ot:ot + 1])
                nc.sync.dma_start(out[ot], o_sb[:])

    nc.finalize()
    return nc


def _prep_inputs(x, gate_w, up_w, down_w, gate_alpha, gate_bias, up_alpha, up_bias,
                 down_alpha, down_bias, down_input_scale):
    bf16 = ml_dtypes.bfloat16

    # per-core activations: xT[hi, ho, t] = x[c*TC + t, ho*128 + hi]
    xTs = []
    for c in range(NCORES):
        xc = np.asarray(x[c * TC:(c + 1) * TC], dtype=np.float32)
        xTs.append(np.ascontiguousarray(
            xc.T.reshape(HT, 128, TC).transpose(1, 0, 2)).astype(bf16))

    # gate/up: w[it, hi, ho, ii] = W[it*128 + ii, ho*128 + hi]
    def prep_gu(w):
        w4 = np.asarray(w, dtype=np.float32).reshape(IT, 128, HT, 128)
        return np.ascontiguousarray(w4.transpose(0, 3, 2, 1)).astype(bf16)

    gw_h = prep_gu(gate_w)
    uw_h = prep_gu(up_w)

    # down: dw[ot, half, ii, k, oi] = down_w[ot*128 + oi, (half*DHALF + k)*128 + ii]
    d5 = np.asarray(down_w, dtype=np.float32).reshape(OT, 128, 2, DHALF, 128)
    dw_h = np.ascontiguousarray(d5.transpose(0, 2, 4, 3, 1)).astype(bf16)

    def perch(v, nt):
        return np.ascontiguousarray(
            np.asarray(v, dtype=np.float64).reshape(nt, 128).T).astype(np.float32)

    s = float(np.asarray(down_input_scale, dtype=np.float64))
    ga_h = perch(gate_alpha, IT)
    gb_h = perch(gate_bias, IT)
    ua_h = perch(np.asarray(up_alpha, dtype=np.float64) / s, IT)
    ub_h = perch(np.asarray(up_bias, dtype=np.float64) / s, IT)
    da_h = perch(down_alpha, OT)
    db_h = perch(down_bias, OT)

    in_maps = []
    for c in range(NCORES):
        in_maps.append(dict(xT=xTs[c], gw=gw_h, uw=uw_h, dw=dw_h,
                            ga=ga_h, gb=gb_h, ua=ua_h, ub=ub_h,
                            da=da_h, db=db_h))
    return in_maps


last_result = None


def kernel(**inputs):
    global _cached_nc, last_result
    if _cached_nc is None:
        _cached_nc = _build()
    in_maps = _prep_inputs(**inputs)
    res = run_bass_kernel_spmd(_cached_nc, in_maps, core_ids=list(range(NCORES)))
    last_result = res
    cols = [res.results[c]["out"].reshape(H, TC) for c in range(NCORES)]
    full = np.concatenate(cols, axis=1)  # [H, T]
    return np.ascontiguousarray(full.T).astype(np.float32)  # [T, H]
